# revision 1
# baseline (speedup 1.0000x reference)
"""MoE (top-2, capacity-dropped) Trainium2 kernel — expert-parallel across 8 NeuronCores.

Strategy
--------
Every core receives the FULL token tensor x and replicates the (cheap) routing
computation in fp32; each core owns one expert (its W1/W2/b1/b2 shard arrives as
per-core inputs). Dispatch is a hardware row-gather (dma_gather transpose mode,
bf16) of the <=2048 capacity-kept tokens; the FFN runs in bf16 on the tensor
engine with fp32 PSUM accumulation; the combine is a hardware scatter-add of the
weighted expert outputs into a token-indexed partial buffer, followed by an
8-core ReduceScatter so each core ends up with the final output for its 1/8
token slice (pure data-parallel output sharding -> host just concatenates).

Routing details (matches the reference exactly, in exact arithmetic):
  - top-2 selection on fp32 logits (softmax is monotonic -> argmax of logits);
  - renormalized weights w1 = sigmoid(l1 - l2), w2 = sigmoid(l2 - l1);
  - capacity keeping via a global cumulative sum over tokens per expert
    (rank-0 assignments counted before rank-1), computed with per-tile scans +
    a tile-offset scan (tensor_tensor_scan), all on-device;
  - slot->token map built with a hardware scatter-add into a DRAM staging table
    and gathered back in slot order.
"""

import numpy as np
import ml_dtypes

import concourse.bass as bass
import concourse.tile as tile
from concourse import bacc, mybir
from concourse.bass_utils import run_bass_kernel_spmd
from concourse.masks import make_identity

F32 = mybir.dt.float32
BF16 = mybir.dt.bfloat16
I16 = mybir.dt.int16
I32 = mybir.dt.int32
AF = mybir.ActivationFunctionType
OP = mybir.AluOpType

P = 128
E = 8
TOPK = 2
B, S, D = 2, 4096, 1024
H = 4096
T = B * S                  # 8192 tokens
C = 2048                   # capacity per expert
NT = T // P                # 64 token tiles
DC = D // P                # 8 d-chunks
HC = H // P                # 32 h-chunks
TRASH_SLOT = 2100          # staging rows >= C collect dropped tokens
STAGE_ROWS = 2176          # 17 * 128
PART_ROWS = 8320           # 65 * 128 (8192 tokens + trash rows)
TRASH_TOK = 8200
SLOT_BLOCKS = [(k * 256, 256) for k in range(8)]


def wrap16_const(n):
    """Host-side: slot indices 0..n-1 in the [16, n/16] wrapped layout, tiled to 128 rows."""
    out = np.zeros((16, n // 16), dtype=np.int16)
    j = np.arange(n)
    out[j % 16, j // 16] = j.astype(np.int16)
    return np.tile(out, (8, 1))


def build_moe(debug=False):
    nc = bacc.Bacc("TRN2", target_bir_lowering=False, debug=False, num_devices=E)

    x_in = nc.dram_tensor("x", [T, D], F32, kind="ExternalInput").ap()
    wg_in = nc.dram_tensor("wg", [P, DC, E], F32, kind="ExternalInput").ap()
    sel_in = nc.dram_tensor("sel", [P, E], F32, kind="ExternalInput").ap()
    w1_in = nc.dram_tensor("w1s", [P, DC, H], BF16, kind="ExternalInput").ap()
    w2_in = nc.dram_tensor("w2s", [P, HC, D], BF16, kind="ExternalInput").ap()
    b1_in = nc.dram_tensor("b1s", [P, HC], F32, kind="ExternalInput").ap()
    b2_in = nc.dram_tensor("b2r", [1, D], BF16, kind="ExternalInput").ap()
    gidx_in = nc.dram_tensor("gidx", [P, C // 16], I16, kind="ExternalInput").ap()

    out_sl = nc.dram_tensor("out_slice", [T // E, D], F32, kind="ExternalOutput").ap()

    xbf_dram = nc.dram_tensor("xbf_stage", [T, D], BF16)
    map_stage = nc.dram_tensor("map_stage", [STAGE_ROWS, 64], F32)
    partial = nc.dram_tensor(
        "partial", [PART_ROWS, D], BF16, kind="ExternalOutput" if debug else "Internal"
    )
    rs_out = nc.dram_tensor("rs_out", [T // E, D], BF16)
    if debug:
        dbg_logits = nc.dram_tensor("dbg_logits", [P, NT, E], F32, kind="ExternalOutput").ap()
        dbg_map = nc.dram_tensor("dbg_map", [P, C // P, 64], F32, kind="ExternalOutput").ap()
        dbg_cw = nc.dram_tensor("dbg_cw", [P, NT], F32, kind="ExternalOutput").ap()
        dbg_pos = nc.dram_tensor("dbg_pos", [P, NT], F32, kind="ExternalOutput").ap()

    with tile.TileContext(nc) as tc:
        with (
            tc.tile_pool(name="const", bufs=1) as const,
            tc.tile_pool(name="persist", bufs=1) as persist,
        ):
            # ---------------- constants ----------------
            ident = const.tile([P, P], F32)
            make_identity(nc, ident[:])
            wg_sb = const.tile([P, DC, E], F32)
            nc.sync.dma_start(wg_sb[:], wg_in[:])
            sel_sb = const.tile([P, E], F32)
            nc.sync.dma_start(sel_sb[:], sel_in[:])
            b1_sb = const.tile([P, HC], F32)
            nc.sync.dma_start(b1_sb[:], b1_in[:])
            b2_sb = const.tile([1, D], BF16)
            nc.sync.dma_start(b2_sb[:], b2_in[:])
            ones1 = const.tile([1, P], BF16)
            nc.vector.memset(ones1[:], 1.0)
            gidx_sb = const.tile([P, C // 16], I16)
            nc.sync.dma_start(gidx_sb[:], gidx_in[:])

            # zero the combine partial buffer + map staging table
            with tc.tile_pool(name="zpool", bufs=1) as zpool:
                zero_bf = zpool.tile([P, D], BF16)
                nc.vector.memset(zero_bf[:], 0.0)
                for i in range(PART_ROWS // P):
                    nc.sync.dma_start(partial[i * P:(i + 1) * P, :], zero_bf[:])
                zero_f32 = zpool.tile([P, (STAGE_ROWS // P) * 64], F32)
                nc.vector.memset(zero_f32[:], 0.0)
                nc.sync.dma_start(
                    map_stage[:].rearrange("(a p) c -> p a c", p=P), zero_f32[:].rearrange("p (a c) -> p a c", c=64)
                )

            logits_sb = persist.tile([P, NT, E], F32)

            # ---------------- phase R1: load x, cast to bf16, transpose, logits ----------------
            with (
                tc.tile_pool(name="r1x", bufs=3) as r1x,
                tc.tile_pool(name="r1xb", bufs=3) as r1xb,
                tc.tile_pool(name="r1xt", bufs=3) as r1xt,
                tc.tile_pool(name="r1pst", bufs=2, space="PSUM") as r1pst,
                tc.tile_pool(name="r1psl", bufs=2, space="PSUM") as r1psl,
            ):
                for i in range(NT):
                    x_sb = r1x.tile([P, D], F32)
                    nc.sync.dma_start(x_sb[:], x_in[i * P:(i + 1) * P, :])
                    xb_sb = r1xb.tile([P, D], BF16)
                    nc.gpsimd.tensor_copy(xb_sb[:], x_sb[:])
                    nc.sync.dma_start(xbf_dram[i * P:(i + 1) * P, :], xb_sb[:])

                    lg_ps = r1psl.tile([P, E], F32, space="PSUM")
                    for half in range(2):
                        tr_ps = r1pst.tile([P, 4 * P], F32, space="PSUM")
                        for j in range(4):
                            dc = half * 4 + j
                            nc.tensor.matmul(
                                tr_ps[:, j * P:(j + 1) * P],
                                x_sb[:, dc * P:(dc + 1) * P],
                                ident[:],
                                is_transpose=True,
                                start=(j == 0),
                                stop=(j == 3),
                            )
                        xt_sb = r1xt.tile([P, 4 * P], F32)
                        nc.vector.tensor_copy(xt_sb[:], tr_ps[:])
                        for j in range(4):
                            dc = half * 4 + j
                            nc.tensor.matmul(
                                lg_ps[:],
                                xt_sb[:, j * P:(j + 1) * P],
                                wg_sb[:, dc, :],
                                start=(dc == 0),
                                stop=(dc == DC - 1),
                            )
                    nc.vector.tensor_copy(logits_sb[:, i, :], lg_ps[:])

            # ---------------- phase R2: top-2 + weights (token-tile layout) ----------------
            with (
                tc.tile_pool(name="r2", bufs=1) as r2,
                tc.tile_pool(name="r3ps", bufs=1, space="PSUM") as r3ps,
            ):
                m1 = r2.tile([P, NT], F32)
                nc.vector.tensor_reduce(m1[:], logits_sb[:], axis=mybir.AxisListType.X, op=OP.max)
                oh1 = r2.tile([P, NT, E], F32)
                nc.vector.tensor_tensor(
                    oh1[:], logits_sb[:], m1[:].rearrange("p t -> p t ()").to_broadcast([P, NT, E]),
                    op=OP.is_equal,
                )
                masked = r2.tile([P, NT, E], F32)
                nc.vector.tensor_scalar(masked[:], oh1[:], -1e9, None, op0=OP.mult)
                nc.vector.tensor_tensor(masked[:], masked[:], logits_sb[:], op=OP.add)
                m2 = r2.tile([P, NT], F32)
                nc.vector.tensor_reduce(m2[:], masked[:], axis=mybir.AxisListType.X, op=OP.max)
                oh2 = r2.tile([P, NT, E], F32)
                nc.vector.tensor_tensor(
                    oh2[:], masked[:], m2[:].rearrange("p t -> p t ()").to_broadcast([P, NT, E]),
                    op=OP.is_equal,
                )
                delta = r2.tile([P, NT], F32)
                nc.vector.tensor_tensor(delta[:], m2[:], m1[:], op=OP.subtract)
                w1 = r2.tile([P, NT], F32)
                nc.scalar.activation(w1[:], delta[:], AF.Sigmoid, scale=-1.0)
                w2 = r2.tile([P, NT], F32)
                nc.scalar.activation(w2[:], delta[:], AF.Sigmoid)

                # select this core's expert column: oh_e = sum_E(oh * sel)
                sel_b = sel_sb[:].rearrange("p e -> p () e").to_broadcast([P, NT, E])
                tmp = r2.tile([P, NT, E], F32)
                oh1e = r2.tile([P, NT], F32)
                nc.vector.tensor_tensor(tmp[:], oh1[:], sel_b, op=OP.mult)
                nc.vector.tensor_reduce(oh1e[:], tmp[:], axis=mybir.AxisListType.X, op=OP.max)
                oh2e = r2.tile([P, NT], F32)
                nc.vector.tensor_tensor(tmp[:], oh2[:], sel_b, op=OP.mult)
                nc.vector.tensor_reduce(oh2e[:], tmp[:], axis=mybir.AxisListType.X, op=OP.max)

                # ---------------- phase R3: capacity cumsum in [tile, token] layout ----------
                # transpose oh1e/oh2e [128, 64] -> [64, 128] (packed into one psum bank)
                ohT_ps = r3ps.tile([P, 2 * P], F32, space="PSUM")
                nc.tensor.matmul(ohT_ps[0:NT, 0:P], oh1e[:], ident[:], is_transpose=True, start=True, stop=False)
                nc.tensor.matmul(ohT_ps[0:NT, P:2 * P], oh2e[:], ident[:], is_transpose=True, start=False, stop=True)
                oh_ic = r2.tile([NT, 2, P], F32)
                nc.vector.tensor_copy(oh_ic[:], ohT_ps[0:NT, :].rearrange("a (k p) -> a k p", k=2))

                ic = r2.tile([NT, 2, P], F32)   # per-tile inclusive cumsums, both ranks
                nc.vector.tensor_tensor_scan(
                    ic[:, 0, :], oh_ic[:, 0, :], oh_ic[:, 0, :], 0.0, op0=OP.add, op1=OP.bypass
                )
                nc.vector.tensor_tensor_scan(
                    ic[:, 1, :], oh_ic[:, 1, :], oh_ic[:, 1, :], 0.0, op0=OP.add, op1=OP.bypass
                )
                # tile totals -> [1, 64] via transpose, prefix-scan, back
                sT_ps = r3ps.tile([P, 2 * NT], F32, space="PSUM")
                nc.tensor.matmul(sT_ps[0:1, 0:NT], ic[:, 0, P - 1:P], ident[0:NT, 0:NT], is_transpose=True, start=True, stop=False)
                nc.tensor.matmul(sT_ps[0:1, NT:2 * NT], ic[:, 1, P - 1:P], ident[0:NT, 0:NT], is_transpose=True, start=False, stop=True)
                sT = r2.tile([1, 2, NT], F32)
                nc.vector.tensor_copy(sT[:], sT_ps[0:1, :].rearrange("a (k t) -> a k t", k=2))
                S1 = r2.tile([1, 2, NT], F32)
                nc.vector.tensor_tensor_scan(
                    S1[:, 0, :], sT[:, 0, :], sT[:, 0, :], 0.0, op0=OP.add, op1=OP.bypass
                )
                c0 = r2.tile([1, 1], F32)
                nc.vector.tensor_scalar(c0[:], S1[:, 0, NT - 1:NT], 2048.0, None, op0=OP.min)
                nc.vector.tensor_tensor_scan(
                    S1[:, 1, :], sT[:, 1, :], sT[:, 1, :], c0[:], op0=OP.add, op1=OP.bypass
                )
                offsT = r2.tile([1, 2, NT], F32)
                nc.vector.tensor_tensor(offsT[:], S1[:], sT[:], op=OP.subtract)
                # back-transpose offsets to [64, 1] per rank
                offs = r2.tile([NT, 2, 1], F32)
                for r in range(2):
                    offs_ps = r3ps.tile([P, 1], F32, space="PSUM", name="offs_ps")
                    nc.tensor.matmul(offs_ps[0:NT, :], offsT[:, r, :], ident[0:1, 0:1], is_transpose=True, start=True, stop=True)
                    nc.vector.tensor_copy(offs[:, r, :], offs_ps[0:NT, :])

                cs = r2.tile([NT, 2, P], F32)
                nc.vector.tensor_scalar(cs[:, 0, :], ic[:, 0, :], offs[:, 0, :], None, op0=OP.add)
                nc.vector.tensor_scalar(cs[:, 1, :], ic[:, 1, :], offs[:, 1, :], None, op0=OP.add)

                keep = r2.tile([NT, 2, P], F32)
                nc.vector.tensor_scalar(keep[:], cs[:], float(C), None, op0=OP.is_le)
                k12 = r2.tile([NT, 2, P], F32)
                nc.vector.tensor_tensor(k12[:], keep[:], oh_ic[:], op=OP.mult)

                # pos = k1*cs1 + k2*cs2 + TRASH + (-1 - TRASH)*(k1+k2)
                kcs = r2.tile([NT, 2, P], F32)
                nc.vector.tensor_tensor(kcs[:], k12[:], cs[:], op=OP.mult)
                pos_ic = r2.tile([NT, P], F32)
                nc.vector.tensor_tensor(pos_ic[:], kcs[:, 0, :], kcs[:, 1, :], op=OP.add)
                ksum = r2.tile([NT, P], F32)
                nc.vector.tensor_tensor(ksum[:], k12[:, 0, :], k12[:, 1, :], op=OP.add)
                nc.vector.tensor_scalar(
                    ksum[:], ksum[:], -float(TRASH_SLOT + 1), float(TRASH_SLOT), op0=OP.mult, op1=OP.add
                )
                nc.vector.tensor_tensor(pos_ic[:], pos_ic[:], ksum[:], op=OP.add)

                # back to token layout: pos [128, 64] (int16) and k1/k2 [128, 64]
                pk_ps = r3ps.tile([P, 3 * NT], F32, space="PSUM")
                nc.tensor.matmul(pk_ps[:, 0:NT], pos_ic[:], ident[0:NT, 0:NT], is_transpose=True, start=True, stop=False)
                nc.tensor.matmul(pk_ps[:, NT:2 * NT], k12[:, 0, :], ident[0:NT, 0:NT], is_transpose=True, start=False, stop=False)
                nc.tensor.matmul(pk_ps[:, 2 * NT:3 * NT], k12[:, 1, :], ident[0:NT, 0:NT], is_transpose=True, start=False, stop=True)
                pos_i16 = r2.tile([P, NT], I16)
                nc.vector.tensor_copy(pos_i16[:], pk_ps[:, 0:NT])
                cw_tok = r2.tile([P, NT], F32)
                t1 = r2.tile([P, NT], F32)
                nc.vector.tensor_tensor(cw_tok[:], w1[:], pk_ps[:, NT:2 * NT], op=OP.mult)
                nc.vector.tensor_tensor(t1[:], w2[:], pk_ps[:, 2 * NT:3 * NT], op=OP.mult)
                nc.vector.tensor_tensor(cw_tok[:], cw_tok[:], t1[:], op=OP.add)
                if debug:
                    nc.sync.dma_start(dbg_cw[:], cw_tok[:])
                    pos_f_dbg = r2.tile([P, NT], F32)
                    nc.vector.tensor_copy(pos_f_dbg[:], pk_ps[:, 0:NT])
                    nc.sync.dma_start(dbg_pos[:], pos_f_dbg[:])

                # ---------------- build wrapped-16 idx for the staging scatter -------------
                idx_pos = persist.tile([P, NT, E], I16)   # [128, 512] wrapped: col = tile*8+g
                sh_pos = r2.tile([P, NT], I16)
                mask = [(i + 16) % 32 for i in range(32)]
                nc.vector.stream_shuffle(sh_pos[:], pos_i16[:], mask)
                for g in range(8):
                    q, lower = g // 2, (g % 2 == 0)
                    src = pos_i16 if lower else sh_pos
                    nc.vector.tensor_copy(idx_pos[0:16, :, g], src[q * 32:q * 32 + 16, :])
                for k in range(1, 8):
                    nc.sync.dma_start(idx_pos[16 * k:16 * (k + 1), :, :], idx_pos[0:16, :, :])

                # staging scatter input: rows [token_id+1, cw, 0...]
                stage_f = r2.tile([P, NT, 64], F32)
                nc.vector.memset(stage_f[:], 0.0)
                ids = r2.tile([P, NT], I32)
                nc.gpsimd.iota(ids[:], pattern=[[P, NT]], base=1, channel_multiplier=1)
                nc.vector.tensor_copy(stage_f[:, :, 0], ids[:])
                nc.vector.tensor_copy(stage_f[:, :, 1], cw_tok[:])
                for k4 in range(4):
                    nc.gpsimd.dma_scatter_add(
                        out_ap=map_stage[:],
                        in_ap=stage_f[:, 16 * k4:16 * (k4 + 1), :],
                        idxs_ap=idx_pos[:, 16 * k4:16 * (k4 + 1), :].rearrange("p a b -> p (a b)"),
                        num_idxs=T // 4, num_idxs_reg=T // 4, elem_size=64,
                    )

            # ---------------- slot-order maps ----------------
            with tc.tile_pool(name="mapb", bufs=1) as mapb:
                map_got = persist.tile([P, C // P, 64], F32)
                for k8 in range(8):
                    nc.gpsimd.dma_gather(
                        out_ap=map_got[:, 2 * k8:2 * (k8 + 1), :],
                        in_ap=map_stage[:],
                        idxs_ap=gidx_sb[:, 16 * k8:16 * (k8 + 1)],
                        num_idxs=C // 8, num_idxs_reg=C // 8, elem_size=64,
                    )
                if debug:
                    nc.sync.dma_start(dbg_map[:], map_got[:])
                tok0 = mapb.tile([P, C // P], F32)
                nc.vector.tensor_scalar(tok0[:], map_got[:, :, 0], -1.0, None, op0=OP.add)
                xg_f = mapb.tile([P, C // P], F32)
                nc.vector.tensor_scalar(xg_f[:], tok0[:], 0.0, None, op0=OP.max)
                neg = mapb.tile([P, C // P], F32)
                nc.vector.tensor_scalar(neg[:], tok0[:], 0.0, None, op0=OP.is_lt)
                sc_f = mapb.tile([P, C // P], F32)
                nc.vector.tensor_scalar(sc_f[:], neg[:], float(TRASH_TOK + 1), None, op0=OP.mult)
                nc.vector.tensor_tensor(sc_f[:], sc_f[:], tok0[:], op=OP.add)
                xg_i = mapb.tile([P, C // P], I16)
                nc.vector.tensor_copy(xg_i[:], xg_f[:])
                sc_i = mapb.tile([P, C // P], I16)
                nc.vector.tensor_copy(sc_i[:], sc_f[:])

                idx_xg = persist.tile([P, C // P, E], I16)
                idx_sc = persist.tile([P, C // P, E], I16)
                mask = [(i + 16) % 32 for i in range(32)]
                sh_xg = mapb.tile([P, C // P], I16)
                nc.vector.stream_shuffle(sh_xg[:], xg_i[:], mask)
                sh_sc = mapb.tile([P, C // P], I16)
                nc.vector.stream_shuffle(sh_sc[:], sc_i[:], mask)
                for g in range(8):
                    q, lower = g // 2, (g % 2 == 0)
                    nc.vector.tensor_copy(idx_xg[0:16, :, g], (xg_i if lower else sh_xg)[q * 32:q * 32 + 16, :])
                    nc.vector.tensor_copy(idx_sc[0:16, :, g], (sc_i if lower else sh_sc)[q * 32:q * 32 + 16, :])
                for k in range(1, 8):
                    nc.sync.dma_start(idx_xg[16 * k:16 * (k + 1), :, :], idx_xg[0:16, :, :])
                    nc.sync.dma_start(idx_sc[16 * k:16 * (k + 1), :, :], idx_sc[0:16, :, :])

            # ---------------- weights + dispatch gather ----------------
            wpool_cm = tc.tile_pool(name="wpool", bufs=1)
            wpool = wpool_cm.__enter__()
            w1_sb = wpool.tile([P, DC, H], BF16)
            nc.sync.dma_start(w1_sb[:], w1_in[:])
            w2_sb = wpool.tile([P, HC, D], BF16)
            nc.sync.dma_start(w2_sb[:], w2_in[:])

            xTe_tiles = []
            for k8 in range(8):
                xTe_k = persist.tile([P, DC, 256], BF16, name=f"xTe{k8}")
                nc.gpsimd.dma_gather(
                    out_ap=xTe_k[:],
                    in_ap=xbf_dram[:],
                    idxs_ap=idx_xg[:, 2 * k8:2 * (k8 + 1), :].rearrange("p a b -> p (a b)"),
                    num_idxs=C // 8, num_idxs_reg=C // 8, elem_size=D, transpose=True,
                )
                xTe_tiles.append(xTe_k)

            # ---------------- FFN ----------------
            with (
                tc.tile_pool(name="hT", bufs=2) as hTp,
                tc.tile_pool(name="ypool", bufs=2) as ypool,
                tc.tile_pool(name="hps", bufs=2, space="PSUM") as hps,
                tc.tile_pool(name="yps", bufs=1, space="PSUM") as yps,
            ):
                for (b0, nb) in SLOT_BLOCKS:
                    ntiles = nb // P
                    y_tiles = [
                        [yps.tile([P, 512], F32, space="PSUM", name=f"y{st}{dg}") for dg in range(2)]
                        for st in range(ntiles)
                    ]
                    for hc in range(HC):
                        h_ps = hps.tile([P, nb], F32, space="PSUM", name="hps")
                        for dc in range(DC):
                            nc.tensor.matmul(
                                h_ps[:],
                                w1_sb[:, dc, hc * P:(hc + 1) * P],
                                xTe_tiles[b0 // 256][:, dc, :],
                                start=(dc == 0),
                                stop=(dc == DC - 1),
                            )
                        h_sb = hTp.tile([P, nb], BF16, name="hsb")
                        nc.scalar.activation(h_sb[:], h_ps[:], AF.Gelu_apprx_tanh, bias=b1_sb[:, hc:hc + 1])
                        for st in range(ntiles):
                            for dg in range(2):
                                nc.tensor.matmul(
                                    y_tiles[st][dg][:],
                                    h_sb[:, st * P:(st + 1) * P],
                                    w2_sb[:, hc, dg * 512:(dg + 1) * 512],
                                    start=(hc == 0),
                                    stop=False,
                                )
                    y_sb = ypool.tile([P, ntiles, D], BF16, name="ysb")
                    for st in range(ntiles):
                        tile_idx = b0 // P + st
                        for dg in range(2):
                            nc.tensor.matmul(
                                y_tiles[st][dg][:],
                                ones1[:],
                                b2_sb[:, dg * 512:(dg + 1) * 512],
                                start=False,
                                stop=True,
                            )
                            nc.scalar.activation(
                                y_sb[:, st, dg * 512:(dg + 1) * 512],
                                y_tiles[st][dg][:],
                                AF.Copy,
                                scale=map_got[:, tile_idx, 1:2],
                            )
                    nc.gpsimd.dma_scatter_add(
                        out_ap=partial[:], in_ap=y_sb[:], idxs_ap=idx_sc[:, b0 // P:(b0 + nb) // P, :].rearrange("p a b -> p (a b)"),
                        num_idxs=nb, num_idxs_reg=nb, elem_size=D,
                    )

            wpool_cm.__exit__(None, None, None)

            # ---------------- combine: ReduceScatter + output ----------------
            if debug:
                nc.sync.dma_start(dbg_logits[:], logits_sb[:])
            else:
                nc.gpsimd.collective_compute(
                    "ReduceScatter",
                    OP.add,
                    replica_groups=[list(range(E))],
                    ins=[partial[0:T, :].opt()],
                    outs=[rs_out[:].opt()],
                )
                with tc.tile_pool(name="outp", bufs=2) as outp:
                    for i in range(T // E // P):
                        t_bf = outp.tile([P, D], BF16)
                        nc.sync.dma_start(t_bf[:], rs_out[i * P:(i + 1) * P, :])
                        t_f = outp.tile([P, D], F32)
                        nc.vector.tensor_copy(t_f[:], t_bf[:])
                        nc.sync.dma_start(out_sl[i * P:(i + 1) * P, :], t_f[:])

    nc.compile()
    return nc


_NC_CACHE = {}


def _get_nc():
    if "nc" not in _NC_CACHE:
        _NC_CACHE["nc"] = build_moe()
    return _NC_CACHE["nc"]


def make_inputs(x, Wg, W1, b1, W2, b2):
    """Host-side sharding: per-core input maps."""
    bf = ml_dtypes.bfloat16
    x = np.ascontiguousarray(np.asarray(x, dtype=np.float32).reshape(T, D))
    wg = np.ascontiguousarray(
        np.asarray(Wg, dtype=np.float32).reshape(DC, P, E).transpose(1, 0, 2)
    )
    gidx = wrap16_const(C)
    in_maps = []
    for e in range(E):
        w1s = np.ascontiguousarray(
            np.asarray(W1[e], dtype=np.float32).reshape(DC, P, H).transpose(1, 0, 2).astype(bf)
        )
        w2s = np.ascontiguousarray(
            np.asarray(W2[e], dtype=np.float32).reshape(HC, P, D).transpose(1, 0, 2).astype(bf)
        )
        b1s = np.ascontiguousarray(np.asarray(b1[e], dtype=np.float32).reshape(HC, P).T)
        b2r = np.asarray(b2[e], dtype=np.float32).reshape(1, D).astype(bf)
        sel = np.zeros((P, E), dtype=np.float32)
        sel[:, e] = 1.0
        in_maps.append({
            "x": x, "wg": wg, "sel": sel,
            "w1s": w1s, "w2s": w2s, "b1s": b1s, "b2r": b2r,
            "gidx": gidx,
        })
    return in_maps


def kernel(x, Wg, W1, b1, W2, b2):
    nc = _get_nc()
    in_maps = make_inputs(x, Wg, W1, b1, W2, b2)
    res = run_bass_kernel_spmd(nc, in_maps, list(range(E)))
    out = np.concatenate([res.results[e]["out_slice"] for e in range(E)], axis=0)
    return out.reshape(B, S, D).astype(np.float32)



# revision 11
# speedup vs baseline: 1.1157x; 1.1157x over previous
"""MoE (top-2, capacity-dropped) Trainium2 kernel — expert-parallel across 8 NeuronCores.

Strategy (v1)
-------------
Routing is data-parallel: each core computes fp32 logits for its 1/8 token
slice (PE transposes + matmul vs Wg), the slices are AllGathered (32 KB each),
and every core then runs the identical top-2 + capacity-cumsum scans so all
cores agree on the routing tables bit-for-bit. Each core owns one expert:
dispatch is a row-granular hardware gather of the <=2048 capacity-kept tokens
straight from the fp32 input (4 KB rows), transposed+cast to bf16 on the
tensor engine. The FFN is software-pipelined (W1 of the next h-chunk is issued
before W2 of the previous one) so the tensor queue never stalls on the scalar
gelu and the PE stays at its max p-state. The combine is a hardware scatter of
weighted expert outputs into a token-indexed partial buffer followed by an
8-core ReduceScatter; each core emits the final fp32 output for its 1/8 token
slice (host concatenates).

Routing math matches the reference exactly in fp32:
  - top-2 on fp32 logits (softmax is monotonic -> argmax of logits);
  - renormalized weights w1 = sigmoid(l1 - l2), w2 = sigmoid(l2 - l1);
  - capacity keeping via global cumulative sums (rank-0 before rank-1) with
    per-tile scans + a tile-offset scan, all on-device;
  - slot->token map built with a scatter-add into a DRAM staging table and
    gathered back in slot order.
"""

import numpy as np
import ml_dtypes

import concourse.bass as bass
import concourse.tile as tile
from concourse import bacc, mybir
from concourse.bass_utils import run_bass_kernel_spmd
from concourse.masks import make_identity

F32 = mybir.dt.float32
BF16 = mybir.dt.bfloat16
I16 = mybir.dt.int16
I32 = mybir.dt.int32
AF = mybir.ActivationFunctionType
OP = mybir.AluOpType

P = 128
E = 8
TOPK = 2
B, S, D = 2, 4096, 1024
H = 4096
T = B * S                  # 8192 tokens
C = 2048                   # capacity per expert
NT = T // P                # 64 token tiles
NTL = NT // E              # 8 token tiles per core (routing slice)
DC = D // P                # 8 d-chunks
HC = H // P                # 32 h-chunks
TRASH_SLOT = 2100          # staging rows >= C collect dropped tokens
STAGE_ROWS = 2176          # 17 * 128
PART_ROWS = 8320           # 65 * 128 (8192 tokens + trash rows)
TRASH_TOK = 8200
NB = 8                     # FFN slot blocks
BS = C // NB               # 256 slots per block


def wrap16_const(n):
    """Host-side: slot indices 0..n-1 in the [16, n/16] wrapped layout, tiled to 128 rows."""
    out = np.zeros((16, n // 16), dtype=np.int16)
    j = np.arange(n)
    out[j % 16, j // 16] = j.astype(np.int16)
    return np.tile(out, (8, 1))


def build_moe(debug=False, dp_logits=True, bulk_on_sync=False):
    nc = bacc.Bacc("TRN2", target_bir_lowering=False, debug=False, num_devices=E)

    x_in = nc.dram_tensor("x", [T, D], F32, kind="ExternalInput").ap()
    xsl_in = nc.dram_tensor("xsl", [T // E, D], F32, kind="ExternalInput").ap()
    wg_in = nc.dram_tensor("wg", [P, DC, E], F32, kind="ExternalInput").ap()
    sel_in = nc.dram_tensor("sel", [P, E], F32, kind="ExternalInput").ap()
    w1_in = nc.dram_tensor("w1s", [P, DC, H], BF16, kind="ExternalInput").ap()
    w2_in = nc.dram_tensor("w2s", [P, HC, D], BF16, kind="ExternalInput").ap()
    b1_in = nc.dram_tensor("b1s", [P, HC], F32, kind="ExternalInput").ap()
    b2_in = nc.dram_tensor("b2r", [1, D], BF16, kind="ExternalInput").ap()
    gidx_in = nc.dram_tensor("gidx", [P, C // 16], I16, kind="ExternalInput").ap()

    out_sl = nc.dram_tensor("out_slice", [T // E, D], F32, kind="ExternalOutput").ap()

    lg_slice = nc.dram_tensor("lg_slice", [P, NTL * E], F32)
    lg_full = nc.dram_tensor("lg_full", [E * P, NTL * E], F32)
    map_stage = nc.dram_tensor("map_stage", [STAGE_ROWS, 64], F32)
    partial = nc.dram_tensor(
        "partial", [PART_ROWS, D], BF16, kind="ExternalOutput" if debug else "Internal"
    )
    rs_out = nc.dram_tensor("rs_out", [T // E, D], BF16)
    if debug:
        dbg_logits = nc.dram_tensor("dbg_logits", [P, NT, E], F32, kind="ExternalOutput").ap()
        dbg_map = nc.dram_tensor("dbg_map", [P, C // P, 64], F32, kind="ExternalOutput").ap()
        dbg_cw = nc.dram_tensor("dbg_cw", [P, NT], F32, kind="ExternalOutput").ap()
        dbg_pos = nc.dram_tensor("dbg_pos", [P, NT], F32, kind="ExternalOutput").ap()

    with tile.TileContext(nc) as tc:
        with (
            tc.tile_pool(name="const", bufs=1) as const,
            tc.tile_pool(name="persist", bufs=1) as persist,
            tc.tile_pool(name="wpool", bufs=1) as wpool,
        ):
            # ---------------- constants (sync queue: small, latency-critical) ----
            ident = const.tile([P, P], F32)
            make_identity(nc, ident[:])
            wg_sb = const.tile([P, DC, E], F32)
            nc.sync.dma_start(wg_sb[:], wg_in[:])
            sel_sb = const.tile([P, E], F32)
            nc.sync.dma_start(sel_sb[:], sel_in[:])
            b1_sb = const.tile([P, HC], F32)
            nc.sync.dma_start(b1_sb[:], b1_in[:])
            b2_sb = const.tile([1, D], BF16)
            nc.sync.dma_start(b2_sb[:], b2_in[:])
            ones1 = const.tile([1, P], BF16)
            nc.vector.memset(ones1[:], 1.0)
            gidx_sb = const.tile([P, C // 16], I16)
            nc.sync.dma_start(gidx_sb[:], gidx_in[:])

            # ------- bulk background DMA (scalar/Activation HWDGE queue) --------
            # order matters: map_stage zero is needed early (before the staging
            # scatter); weights before FFN; partial zero before first y-scatter.
            bulk = nc.sync if bulk_on_sync else nc.scalar
            with tc.tile_pool(name="zpool", bufs=1) as zpool:
                zero_f32 = zpool.tile([P, (STAGE_ROWS // P) * 64], F32)
                nc.vector.memset(zero_f32[:], 0.0)
                bulk.dma_start(
                    map_stage[:].rearrange("(a p) c -> p a c", p=P),
                    zero_f32[:].rearrange("p (a c) -> p a c", c=64),
                )
                w1_sb = wpool.tile([P, DC, H], BF16)
                bulk.dma_start(w1_sb[:], w1_in[:])
                w2_sb = wpool.tile([P, HC, D], BF16)
                bulk.dma_start(w2_sb[:], w2_in[:])
                zero_bf = zpool.tile([P, D], BF16)
                nc.vector.memset(zero_bf[:], 0.0)
                for i in range(PART_ROWS // P):
                    bulk.dma_start(partial[i * P:(i + 1) * P, :], zero_bf[:])

            logits_sb = persist.tile([P, NT, E], F32)

            # ---------------- phase R1: data-parallel logits over 1/8 tokens ----
            with (
                tc.tile_pool(name="r1x", bufs=3) as r1x,
                tc.tile_pool(name="r1xt", bufs=3) as r1xt,
                tc.tile_pool(name="r1pst", bufs=3, space="PSUM") as r1pst,
                tc.tile_pool(name="r1psl", bufs=2, space="PSUM") as r1psl,
            ):
                ntiles = NTL if dp_logits else NT
                xsrc = xsl_in if dp_logits else x_in
                if dp_logits:
                    lgT_sb = persist.tile([P, ntiles, E], F32, name="lgT_sb")
                else:
                    lgT_sb = logits_sb
                for i in range(ntiles):
                    x_sb = r1x.tile([P, D], F32)
                    nc.sync.dma_start(x_sb[:], xsrc[i * P:(i + 1) * P, :])
                    lg_ps = r1psl.tile([P, E], F32, space="PSUM")
                    for half in range(2):
                        tr_ps = r1pst.tile([P, 4 * P], F32, space="PSUM")
                        for j in range(4):
                            dc = half * 4 + j
                            nc.tensor.matmul(
                                tr_ps[:, j * P:(j + 1) * P],
                                x_sb[:, dc * P:(dc + 1) * P],
                                ident[:],
                                is_transpose=True,
                                start=(j == 0),
                                stop=(j == 3),
                            )
                        xt_sb = r1xt.tile([P, 4 * P], F32)
                        nc.vector.tensor_copy(xt_sb[:], tr_ps[:])
                        for j in range(4):
                            dc = half * 4 + j
                            nc.tensor.matmul(
                                lg_ps[:],
                                xt_sb[:, j * P:(j + 1) * P],
                                wg_sb[:, dc, :],
                                start=(dc == 0),
                                stop=(dc == DC - 1),
                            )
                    nc.vector.tensor_copy(lgT_sb[:, i, :], lg_ps[:])
                if dp_logits:
                    nc.sync.dma_start(
                        lg_slice[:], lgT_sb[:].rearrange("p a e -> p (a e)")
                    )

            # ---------------- logits exchange + reload --------------------------
            if dp_logits:
                nc.gpsimd.collective_compute(
                    "AllGather",
                    OP.bypass,
                    replica_groups=[list(range(E))],
                    ins=[lg_slice[:].opt()],
                    outs=[lg_full[:].opt()],
                )
                nc.sync.dma_start(
                    logits_sb[:].rearrange("p (j a) e -> p j (a e)", j=E),
                    lg_full[:].rearrange("(j p) c -> p j c", p=P),
                )
            if debug:
                nc.sync.dma_start(dbg_logits[:], logits_sb[:])

            # ---------------- phase R2: top-2 + weights (token-tile layout) -----
            with (
                tc.tile_pool(name="r2", bufs=1) as r2,
                tc.tile_pool(name="r3ps", bufs=1, space="PSUM") as r3ps,
            ):
                m1 = r2.tile([P, NT], F32)
                nc.vector.tensor_reduce(m1[:], logits_sb[:], axis=mybir.AxisListType.X, op=OP.max)
                oh1 = r2.tile([P, NT, E], F32)
                nc.vector.tensor_tensor(
                    oh1[:], logits_sb[:], m1[:].rearrange("p t -> p t ()").to_broadcast([P, NT, E]),
                    op=OP.is_equal,
                )
                masked = r2.tile([P, NT, E], F32)
                nc.vector.tensor_scalar(masked[:], oh1[:], -1e9, None, op0=OP.mult)
                nc.vector.tensor_tensor(masked[:], masked[:], logits_sb[:], op=OP.add)
                m2 = r2.tile([P, NT], F32)
                nc.vector.tensor_reduce(m2[:], masked[:], axis=mybir.AxisListType.X, op=OP.max)
                oh2 = r2.tile([P, NT, E], F32)
                nc.vector.tensor_tensor(
                    oh2[:], masked[:], m2[:].rearrange("p t -> p t ()").to_broadcast([P, NT, E]),
                    op=OP.is_equal,
                )
                delta = r2.tile([P, NT], F32)
                nc.vector.tensor_tensor(delta[:], m2[:], m1[:], op=OP.subtract)
                w1 = r2.tile([P, NT], F32)
                nc.scalar.activation(w1[:], delta[:], AF.Sigmoid, scale=-1.0)
                w2 = r2.tile([P, NT], F32)
                nc.scalar.activation(w2[:], delta[:], AF.Sigmoid)

                # select this core's expert column: oh_e = sum_E(oh * sel)
                sel_b = sel_sb[:].rearrange("p e -> p () e").to_broadcast([P, NT, E])
                tmp = r2.tile([P, NT, E], F32)
                oh1e = r2.tile([P, NT], F32)
                nc.vector.tensor_tensor(tmp[:], oh1[:], sel_b, op=OP.mult)
                nc.vector.tensor_reduce(oh1e[:], tmp[:], axis=mybir.AxisListType.X, op=OP.max)
                oh2e = r2.tile([P, NT], F32)
                nc.vector.tensor_tensor(tmp[:], oh2[:], sel_b, op=OP.mult)
                nc.vector.tensor_reduce(oh2e[:], tmp[:], axis=mybir.AxisListType.X, op=OP.max)

                # ---------------- phase R3: capacity cumsum in [tile, token] layout
                ohT_ps = r3ps.tile([P, 2 * P], F32, space="PSUM")
                nc.tensor.matmul(ohT_ps[0:NT, 0:P], oh1e[:], ident[:], is_transpose=True, start=True, stop=False)
                nc.tensor.matmul(ohT_ps[0:NT, P:2 * P], oh2e[:], ident[:], is_transpose=True, start=False, stop=True)
                oh_ic = r2.tile([NT, 2, P], F32)
                nc.vector.tensor_copy(oh_ic[:], ohT_ps[0:NT, :].rearrange("a (k p) -> a k p", k=2))

                ic = r2.tile([NT, 2, P], F32)   # per-tile inclusive cumsums, both ranks
                nc.vector.tensor_tensor_scan(
                    ic[:, 0, :], oh_ic[:, 0, :], oh_ic[:, 0, :], 0.0, op0=OP.add, op1=OP.bypass
                )
                nc.vector.tensor_tensor_scan(
                    ic[:, 1, :], oh_ic[:, 1, :], oh_ic[:, 1, :], 0.0, op0=OP.add, op1=OP.bypass
                )
                # tile totals -> [1, 64] via transpose, prefix-scan, back
                sT_ps = r3ps.tile([P, 2 * NT], F32, space="PSUM")
                nc.tensor.matmul(sT_ps[0:1, 0:NT], ic[:, 0, P - 1:P], ident[0:NT, 0:NT], is_transpose=True, start=True, stop=False)
                nc.tensor.matmul(sT_ps[0:1, NT:2 * NT], ic[:, 1, P - 1:P], ident[0:NT, 0:NT], is_transpose=True, start=False, stop=True)
                sT = r2.tile([1, 2, NT], F32)
                nc.vector.tensor_copy(sT[:], sT_ps[0:1, :].rearrange("a (k t) -> a k t", k=2))
                S1 = r2.tile([1, 2, NT], F32)
                nc.vector.tensor_tensor_scan(
                    S1[:, 0, :], sT[:, 0, :], sT[:, 0, :], 0.0, op0=OP.add, op1=OP.bypass
                )
                c0 = r2.tile([1, 1], F32)
                nc.vector.tensor_scalar(c0[:], S1[:, 0, NT - 1:NT], 2048.0, None, op0=OP.min)
                nc.vector.tensor_tensor_scan(
                    S1[:, 1, :], sT[:, 1, :], sT[:, 1, :], c0[:], op0=OP.add, op1=OP.bypass
                )
                offsT = r2.tile([1, 2, NT], F32)
                nc.vector.tensor_tensor(offsT[:], S1[:], sT[:], op=OP.subtract)
                # back-transpose offsets to [64, 1] per rank
                offs = r2.tile([NT, 2, 1], F32)
                for r in range(2):
                    offs_ps = r3ps.tile([P, 1], F32, space="PSUM", name="offs_ps")
                    nc.tensor.matmul(offs_ps[0:NT, :], offsT[:, r, :], ident[0:1, 0:1], is_transpose=True, start=True, stop=True)
                    nc.vector.tensor_copy(offs[:, r, :], offs_ps[0:NT, :])

                cs = r2.tile([NT, 2, P], F32)
                nc.vector.tensor_scalar(cs[:, 0, :], ic[:, 0, :], offs[:, 0, :], None, op0=OP.add)
                nc.vector.tensor_scalar(cs[:, 1, :], ic[:, 1, :], offs[:, 1, :], None, op0=OP.add)

                keep = r2.tile([NT, 2, P], F32)
                nc.vector.tensor_scalar(keep[:], cs[:], float(C), None, op0=OP.is_le)
                k12 = r2.tile([NT, 2, P], F32)
                nc.vector.tensor_tensor(k12[:], keep[:], oh_ic[:], op=OP.mult)

                # pos = k1*cs1 + k2*cs2 + TRASH + (-1 - TRASH)*(k1+k2)
                kcs = r2.tile([NT, 2, P], F32)
                nc.vector.tensor_tensor(kcs[:], k12[:], cs[:], op=OP.mult)
                pos_ic = r2.tile([NT, P], F32)
                nc.vector.tensor_tensor(pos_ic[:], kcs[:, 0, :], kcs[:, 1, :], op=OP.add)
                ksum = r2.tile([NT, P], F32)
                nc.vector.tensor_tensor(ksum[:], k12[:, 0, :], k12[:, 1, :], op=OP.add)
                nc.vector.tensor_scalar(
                    ksum[:], ksum[:], -float(TRASH_SLOT + 1), float(TRASH_SLOT), op0=OP.mult, op1=OP.add
                )
                nc.vector.tensor_tensor(pos_ic[:], pos_ic[:], ksum[:], op=OP.add)

                # back to token layout: pos [128, 64] (int16) and k1/k2 [128, 64]
                pk_ps = r3ps.tile([P, 3 * NT], F32, space="PSUM")
                nc.tensor.matmul(pk_ps[:, 0:NT], pos_ic[:], ident[0:NT, 0:NT], is_transpose=True, start=True, stop=False)
                nc.tensor.matmul(pk_ps[:, NT:2 * NT], k12[:, 0, :], ident[0:NT, 0:NT], is_transpose=True, start=False, stop=False)
                nc.tensor.matmul(pk_ps[:, 2 * NT:3 * NT], k12[:, 1, :], ident[0:NT, 0:NT], is_transpose=True, start=False, stop=True)
                pos_i16 = r2.tile([P, NT], I16)
                nc.vector.tensor_copy(pos_i16[:], pk_ps[:, 0:NT])
                cw_tok = r2.tile([P, NT], F32)
                t1 = r2.tile([P, NT], F32)
                nc.vector.tensor_tensor(cw_tok[:], w1[:], pk_ps[:, NT:2 * NT], op=OP.mult)
                nc.vector.tensor_tensor(t1[:], w2[:], pk_ps[:, 2 * NT:3 * NT], op=OP.mult)
                nc.vector.tensor_tensor(cw_tok[:], cw_tok[:], t1[:], op=OP.add)
                if debug:
                    nc.sync.dma_start(dbg_cw[:], cw_tok[:])
                    pos_f_dbg = r2.tile([P, NT], F32)
                    nc.vector.tensor_copy(pos_f_dbg[:], pk_ps[:, 0:NT])
                    nc.sync.dma_start(dbg_pos[:], pos_f_dbg[:])

                # ---------------- build wrapped-16 idx for the staging scatter ----
                idx_pos = persist.tile([P, NT, E], I16)   # [128, 512] wrapped: col = tile*8+g
                sh_pos = r2.tile([P, NT], I16)
                mask = [(i + 16) % 32 for i in range(32)]
                nc.vector.stream_shuffle(sh_pos[:], pos_i16[:], mask)
                for g in range(8):
                    q, lower = g // 2, (g % 2 == 0)
                    src = pos_i16 if lower else sh_pos
                    nc.vector.tensor_copy(idx_pos[0:16, :, g], src[q * 32:q * 32 + 16, :])
                for k in range(1, 8):
                    nc.sync.dma_start(idx_pos[16 * k:16 * (k + 1), :, :], idx_pos[0:16, :, :])

                # staging scatter input: rows [token_id+1, cw, 0...]
                stage_f = r2.tile([P, NT, 64], F32)
                nc.vector.memset(stage_f[:], 0.0)
                ids = r2.tile([P, NT], I32)
                nc.gpsimd.iota(ids[:], pattern=[[P, NT]], base=1, channel_multiplier=1)
                nc.vector.tensor_copy(stage_f[:, :, 0], ids[:])
                nc.vector.tensor_copy(stage_f[:, :, 1], cw_tok[:])
                for k4 in range(4):
                    nc.gpsimd.dma_scatter_add(
                        out_ap=map_stage[:],
                        in_ap=stage_f[:, 16 * k4:16 * (k4 + 1), :],
                        idxs_ap=idx_pos[:, 16 * k4:16 * (k4 + 1), :].rearrange("p a b -> p (a b)"),
                        num_idxs=T // 4, num_idxs_reg=T // 4, elem_size=64,
                    )

            # ---------------- slot-order maps ----------------
            with tc.tile_pool(name="mapb", bufs=1) as mapb:
                map_got = persist.tile([P, C // P, 64], F32)
                # NOTE: dma_gather with num_idxs=2048 in one call faults the
                # device (ucode limit) — keep per-call idx count at 256.
                for k8 in range(8):
                    nc.gpsimd.dma_gather(
                        out_ap=map_got[:, 2 * k8:2 * (k8 + 1), :],
                        in_ap=map_stage[:],
                        idxs_ap=gidx_sb[:, 16 * k8:16 * (k8 + 1)],
                        num_idxs=C // 8, num_idxs_reg=C // 8, elem_size=64,
                    )
                if debug:
                    nc.sync.dma_start(dbg_map[:], map_got[:])
                tok0 = mapb.tile([P, C // P], F32)
                nc.vector.tensor_scalar(tok0[:], map_got[:, :, 0], -1.0, None, op0=OP.add)
                xg_f = mapb.tile([P, C // P], F32)
                nc.vector.tensor_scalar(xg_f[:], tok0[:], 0.0, None, op0=OP.max)
                neg = mapb.tile([P, C // P], F32)
                nc.vector.tensor_scalar(neg[:], tok0[:], 0.0, None, op0=OP.is_lt)
                sc_f = mapb.tile([P, C // P], F32)
                nc.vector.tensor_scalar(sc_f[:], neg[:], float(TRASH_TOK + 1), None, op0=OP.mult)
                nc.vector.tensor_tensor(sc_f[:], sc_f[:], tok0[:], op=OP.add)
                xg_i = mapb.tile([P, C // P], I16)
                nc.vector.tensor_copy(xg_i[:], xg_f[:])
                sc_i = mapb.tile([P, C // P], I16)
                nc.vector.tensor_copy(sc_i[:], sc_f[:])

                idx_xg = persist.tile([P, C // P, E], I16)
                idx_sc = persist.tile([P, C // P, E], I16)
                mask = [(i + 16) % 32 for i in range(32)]
                sh_xg = mapb.tile([P, C // P], I16)
                nc.vector.stream_shuffle(sh_xg[:], xg_i[:], mask)
                sh_sc = mapb.tile([P, C // P], I16)
                nc.vector.stream_shuffle(sh_sc[:], sc_i[:], mask)
                for g in range(8):
                    q, lower = g // 2, (g % 2 == 0)
                    nc.vector.tensor_copy(idx_xg[0:16, :, g], (xg_i if lower else sh_xg)[q * 32:q * 32 + 16, :])
                    nc.vector.tensor_copy(idx_sc[0:16, :, g], (sc_i if lower else sh_sc)[q * 32:q * 32 + 16, :])
                for k in range(1, 8):
                    nc.sync.dma_start(idx_xg[16 * k:16 * (k + 1), :, :], idx_xg[0:16, :, :])
                    nc.sync.dma_start(idx_sc[16 * k:16 * (k + 1), :, :], idx_sc[0:16, :, :])

            # ---------------- dispatch (row gather + PE transpose) + FFN --------
            with (
                tc.tile_pool(name="gpool", bufs=2) as gpool,
                tc.tile_pool(name="xTep", bufs=2) as xTep,
                tc.tile_pool(name="tpps", bufs=2, space="PSUM") as tpps,
                tc.tile_pool(name="hT", bufs=3) as hTp,
                tc.tile_pool(name="ypool", bufs=2) as ypool,
                tc.tile_pool(name="hps", bufs=2, space="PSUM") as hps,
                tc.tile_pool(name="yps", bufs=1, space="PSUM") as yps,
            ):
                def emit_gather(b):
                    xg = gpool.tile([P, 2, D], F32, name="xg")
                    nc.gpsimd.dma_gather(
                        out_ap=xg[:],
                        in_ap=x_in[:],
                        idxs_ap=idx_xg[:, 2 * b:2 * (b + 1), :].rearrange("p a b -> p (a b)"),
                        num_idxs=BS, num_idxs_reg=BS, elem_size=D,
                    )
                    return xg

                def emit_transpose(xg):
                    xTe = xTep.tile([P, DC, BS], BF16, name="xTe")
                    for st in range(2):
                        for gg in range(2):
                            tp = tpps.tile([P, 4, P], F32, space="PSUM", name="tp")
                            for j in range(4):
                                dc = gg * 4 + j
                                nc.tensor.matmul(
                                    tp[:, j, :],
                                    xg[:, st, dc * P:(dc + 1) * P],
                                    ident[:],
                                    is_transpose=True,
                                    start=(j == 0),
                                    stop=(j == 3),
                                )
                            nc.vector.tensor_copy(
                                xTe[:, gg * 4:(gg + 1) * 4, st * P:(st + 1) * P], tp[:]
                            )
                    return xTe

                xg_tiles = {0: emit_gather(0), 1: emit_gather(1)}
                xTe_cur = emit_transpose(xg_tiles.pop(0))

                for b in range(NB):
                    if b + 2 < NB:
                        xg_tiles[b + 2] = emit_gather(b + 2)
                    xTe_next = emit_transpose(xg_tiles.pop(b + 1)) if b + 1 < NB else None

                    y_tiles = [
                        [yps.tile([P, 512], F32, space="PSUM", name=f"y{st}{dg}") for dg in range(2)]
                        for st in range(2)
                    ]
                    h_prev = None
                    for hc in range(HC):
                        h_ps = hps.tile([P, BS], F32, space="PSUM", name="hps")
                        for dc in range(DC):
                            nc.tensor.matmul(
                                h_ps[:],
                                w1_sb[:, dc, hc * P:(hc + 1) * P],
                                xTe_cur[:, dc, :],
                                start=(dc == 0),
                                stop=(dc == DC - 1),
                            )
                        h_sb = hTp.tile([P, BS], BF16, name="hsb")
                        nc.scalar.activation(h_sb[:], h_ps[:], AF.Gelu_apprx_tanh, bias=b1_sb[:, hc:hc + 1])
                        if h_prev is not None:
                            ph, phc = h_prev
                            for st in range(2):
                                for dg in range(2):
                                    nc.tensor.matmul(
                                        y_tiles[st][dg][:],
                                        ph[:, st * P:(st + 1) * P],
                                        w2_sb[:, phc, dg * 512:(dg + 1) * 512],
                                        start=(phc == 0),
                                        stop=False,
                                    )
                        h_prev = (h_sb, hc)
                    ph, phc = h_prev
                    for st in range(2):
                        for dg in range(2):
                            nc.tensor.matmul(
                                y_tiles[st][dg][:],
                                ph[:, st * P:(st + 1) * P],
                                w2_sb[:, phc, dg * 512:(dg + 1) * 512],
                                start=False,
                                stop=False,
                            )
                    # bias add via ones-outer-product, then weighted drain
                    y_sb = ypool.tile([P, 2, D], BF16, name="ysb")
                    for st in range(2):
                        tile_idx = b * 2 + st
                        for dg in range(2):
                            nc.tensor.matmul(
                                y_tiles[st][dg][:],
                                ones1[:],
                                b2_sb[:, dg * 512:(dg + 1) * 512],
                                start=False,
                                stop=True,
                            )
                            if st == 0:
                                nc.scalar.activation(
                                    y_sb[:, st, dg * 512:(dg + 1) * 512],
                                    y_tiles[st][dg][:],
                                    AF.Copy,
                                    scale=map_got[:, tile_idx, 1:2],
                                )
                            else:
                                nc.vector.tensor_scalar(
                                    y_sb[:, st, dg * 512:(dg + 1) * 512],
                                    y_tiles[st][dg][:],
                                    map_got[:, tile_idx, 1:2],
                                    None,
                                    op0=OP.mult,
                                )
                    nc.gpsimd.dma_scatter_add(
                        out_ap=partial[:], in_ap=y_sb[:],
                        idxs_ap=idx_sc[:, 2 * b:2 * (b + 1), :].rearrange("p a b -> p (a b)"),
                        num_idxs=BS, num_idxs_reg=BS, elem_size=D,
                    )
                    xTe_cur = xTe_next

            # ---------------- combine: ReduceScatter + output ----------------
            nc.gpsimd.collective_compute(
                "ReduceScatter",
                OP.add,
                replica_groups=[list(range(E))],
                ins=[partial[0:T, :].opt()],
                outs=[rs_out[:].opt()],
            )
            with tc.tile_pool(name="outp", bufs=2) as outp:
                for i in range(T // E // P):
                    t_bf = outp.tile([P, D], BF16)
                    nc.sync.dma_start(t_bf[:], rs_out[i * P:(i + 1) * P, :])
                    t_f = outp.tile([P, D], F32)
                    nc.vector.tensor_copy(t_f[:], t_bf[:])
                    nc.sync.dma_start(out_sl[i * P:(i + 1) * P, :], t_f[:])

    nc.compile()
    return nc


_NC_CACHE = {}


def _get_nc():
    if "nc" not in _NC_CACHE:
        _NC_CACHE["nc"] = build_moe()
    return _NC_CACHE["nc"]


def make_inputs(x, Wg, W1, b1, W2, b2):
    """Host-side sharding: per-core input maps."""
    bf = ml_dtypes.bfloat16
    x = np.ascontiguousarray(np.asarray(x, dtype=np.float32).reshape(T, D))
    wg = np.ascontiguousarray(
        np.asarray(Wg, dtype=np.float32).reshape(DC, P, E).transpose(1, 0, 2)
    )
    gidx = wrap16_const(C)
    in_maps = []
    for e in range(E):
        w1s = np.ascontiguousarray(
            np.asarray(W1[e], dtype=np.float32).reshape(DC, P, H).transpose(1, 0, 2).astype(bf)
        )
        w2s = np.ascontiguousarray(
            np.asarray(W2[e], dtype=np.float32).reshape(HC, P, D).transpose(1, 0, 2).astype(bf)
        )
        b1s = np.ascontiguousarray(np.asarray(b1[e], dtype=np.float32).reshape(HC, P).T)
        b2r = np.asarray(b2[e], dtype=np.float32).reshape(1, D).astype(bf)
        sel = np.zeros((P, E), dtype=np.float32)
        sel[:, e] = 1.0
        in_maps.append({
            "x": x, "xsl": x[e * (T // E):(e + 1) * (T // E)], "wg": wg, "sel": sel,
            "w1s": w1s, "w2s": w2s, "b1s": b1s, "b2r": b2r,
            "gidx": gidx,
        })
    return in_maps


def kernel(x, Wg, W1, b1, W2, b2):
    nc = _get_nc()
    in_maps = make_inputs(x, Wg, W1, b1, W2, b2)
    res = run_bass_kernel_spmd(nc, in_maps, list(range(E)))
    out = np.concatenate([res.results[e]["out_slice"] for e in range(E)], axis=0)
    return out.reshape(B, S, D).astype(np.float32)


# revision 14
# speedup vs baseline: 13.6582x; 12.2418x over previous
"""MoE (top-2, capacity-dropped) Trainium2 kernel — expert-parallel across 8 NeuronCores.

Strategy (v1)
-------------
Routing is data-parallel: each core computes fp32 logits for its 1/8 token
slice (PE transposes + matmul vs Wg), the slices are AllGathered (32 KB each),
and every core then runs the identical top-2 + capacity-cumsum scans so all
cores agree on the routing tables bit-for-bit. Each core owns one expert:
dispatch is a row-granular hardware gather of the <=2048 capacity-kept tokens
straight from the fp32 input (4 KB rows), transposed+cast to bf16 on the
tensor engine. The FFN is software-pipelined (W1 of the next h-chunk is issued
before W2 of the previous one) so the tensor queue never stalls on the scalar
gelu and the PE stays at its max p-state. The combine is a hardware scatter of
weighted expert outputs into a token-indexed partial buffer followed by an
8-core ReduceScatter; each core emits the final fp32 output for its 1/8 token
slice (host concatenates).

Routing math matches the reference exactly in fp32:
  - top-2 on fp32 logits (softmax is monotonic -> argmax of logits);
  - renormalized weights w1 = sigmoid(l1 - l2), w2 = sigmoid(l2 - l1);
  - capacity keeping via global cumulative sums (rank-0 before rank-1) with
    per-tile scans + a tile-offset scan, all on-device;
  - slot->token map built with a scatter-add into a DRAM staging table and
    gathered back in slot order.
"""

import numpy as np
import ml_dtypes

import concourse.bass as bass
import concourse.tile as tile
from concourse import bacc, mybir
from concourse.bass_utils import run_bass_kernel_spmd
from concourse.masks import make_identity

F32 = mybir.dt.float32
BF16 = mybir.dt.bfloat16
I16 = mybir.dt.int16
I32 = mybir.dt.int32
AF = mybir.ActivationFunctionType
OP = mybir.AluOpType

P = 128
E = 8
TOPK = 2
B, S, D = 2, 4096, 1024
H = 4096
T = B * S                  # 8192 tokens
C = 2048                   # capacity per expert
NT = T // P                # 64 token tiles
NTL = NT // E              # 8 token tiles per core (routing slice)
DC = D // P                # 8 d-chunks
HC = H // P                # 32 h-chunks
TRASH_SLOT = 2100          # staging rows >= C collect dropped tokens
STAGE_ROWS = 2176          # 17 * 128
PART_ROWS = 8320           # 65 * 128 (8192 tokens + trash rows)
TRASH_TOK = 8200
NB = 8                     # FFN slot blocks
BS = C // NB               # 256 slots per block


def wrap16_const(n):
    """Host-side: slot indices 0..n-1 in the [16, n/16] wrapped layout, tiled to 128 rows."""
    out = np.zeros((16, n // 16), dtype=np.int16)
    j = np.arange(n)
    out[j % 16, j // 16] = j.astype(np.int16)
    return np.tile(out, (8, 1))


def build_moe(debug=False, dp_logits=True, bulk_on_sync=False):
    nc = bacc.Bacc("TRN2", target_bir_lowering=False, debug=False, num_devices=E,
                   num_swdge_queues=4)

    x_in = nc.dram_tensor("x", [T, D], F32, kind="ExternalInput").ap()
    xsl_in = nc.dram_tensor("xsl", [T // E, D], F32, kind="ExternalInput").ap()
    wg_in = nc.dram_tensor("wg", [P, DC, E], F32, kind="ExternalInput").ap()
    sel_in = nc.dram_tensor("sel", [P, E], F32, kind="ExternalInput").ap()
    w1_in = nc.dram_tensor("w1s", [P, DC, H], BF16, kind="ExternalInput").ap()
    w2_in = nc.dram_tensor("w2s", [P, HC, D], BF16, kind="ExternalInput").ap()
    b1_in = nc.dram_tensor("b1s", [P, HC], F32, kind="ExternalInput").ap()
    b2_in = nc.dram_tensor("b2r", [1, D], BF16, kind="ExternalInput").ap()
    gidx_in = nc.dram_tensor("gidx", [P, C // 16], I16, kind="ExternalInput").ap()

    out_sl = nc.dram_tensor("out_slice", [T // E, D], F32, kind="ExternalOutput").ap()

    lg_slice = nc.dram_tensor("lg_slice", [P, NTL * E], F32)
    lg_full = nc.dram_tensor("lg_full", [E * P, NTL * E], F32)
    map_stage = nc.dram_tensor("map_stage", [STAGE_ROWS, 64], F32)
    partial = nc.dram_tensor(
        "partial", [PART_ROWS, D], BF16, kind="ExternalOutput" if debug else "Internal"
    )
    rs_out = nc.dram_tensor("rs_out", [T // E, D], BF16)
    if debug:
        dbg_logits = nc.dram_tensor("dbg_logits", [P, NT, E], F32, kind="ExternalOutput").ap()
        dbg_map = nc.dram_tensor("dbg_map", [P, C // P, 64], F32, kind="ExternalOutput").ap()
        dbg_cw = nc.dram_tensor("dbg_cw", [P, NT], F32, kind="ExternalOutput").ap()
        dbg_pos = nc.dram_tensor("dbg_pos", [P, NT], F32, kind="ExternalOutput").ap()

    def next_q():
        return 0  # placeholder; fixed up post-schedule to lane % 4

    with tile.TileContext(nc) as tc:
        with (
            tc.tile_pool(name="const", bufs=1) as const,
            tc.tile_pool(name="persist", bufs=1) as persist,
            tc.tile_pool(name="wpool", bufs=1) as wpool,
        ):
            # ---------------- constants (sync queue: small, latency-critical) ----
            ident = const.tile([P, P], F32)
            make_identity(nc, ident[:])
            wg_sb = const.tile([P, DC, E], F32)
            nc.sync.dma_start(wg_sb[:], wg_in[:])
            sel_sb = const.tile([P, E], F32)
            nc.sync.dma_start(sel_sb[:], sel_in[:])
            b1_sb = const.tile([P, HC], F32)
            nc.sync.dma_start(b1_sb[:], b1_in[:])
            b2_sb = const.tile([1, D], BF16)
            nc.sync.dma_start(b2_sb[:], b2_in[:])
            ones1 = const.tile([1, P], BF16)
            nc.vector.memset(ones1[:], 1.0)
            gidx_sb = const.tile([P, C // 16], I16)
            nc.sync.dma_start(gidx_sb[:], gidx_in[:])

            # ------- bulk background DMA (scalar/Activation HWDGE queue) --------
            # order matters: map_stage zero is needed early (before the staging
            # scatter); weights before FFN; partial zero before first y-scatter.
            bulk = nc.sync if bulk_on_sync else nc.scalar
            with tc.tile_pool(name="zpool", bufs=1) as zpool:
                zero_f32 = zpool.tile([P, (STAGE_ROWS // P) * 64], F32)
                nc.vector.memset(zero_f32[:], 0.0)
                bulk.dma_start(
                    map_stage[:].rearrange("(a p) c -> p a c", p=P),
                    zero_f32[:].rearrange("p (a c) -> p a c", c=64),
                )
                w1_sb = wpool.tile([P, DC, H], BF16)
                bulk.dma_start(w1_sb[:], w1_in[:])
                w2_sb = wpool.tile([P, HC, D], BF16)
                bulk.dma_start(w2_sb[:], w2_in[:])
                zero_bf = zpool.tile([P, D], BF16)
                nc.vector.memset(zero_bf[:], 0.0)
                for i in range(PART_ROWS // P):
                    bulk.dma_start(partial[i * P:(i + 1) * P, :], zero_bf[:])

            logits_sb = persist.tile([P, NT, E], F32)

            # ---------------- phase R1: data-parallel logits over 1/8 tokens ----
            with (
                tc.tile_pool(name="r1x", bufs=3) as r1x,
                tc.tile_pool(name="r1xt", bufs=3) as r1xt,
                tc.tile_pool(name="r1pst", bufs=3, space="PSUM") as r1pst,
                tc.tile_pool(name="r1psl", bufs=2, space="PSUM") as r1psl,
            ):
                ntiles = NTL if dp_logits else NT
                xsrc = xsl_in if dp_logits else x_in
                if dp_logits:
                    lgT_sb = persist.tile([P, ntiles, E], F32, name="lgT_sb")
                else:
                    lgT_sb = logits_sb
                for i in range(ntiles):
                    x_sb = r1x.tile([P, D], F32)
                    nc.sync.dma_start(x_sb[:], xsrc[i * P:(i + 1) * P, :])
                    lg_ps = r1psl.tile([P, E], F32, space="PSUM")
                    for half in range(2):
                        tr_ps = r1pst.tile([P, 4 * P], F32, space="PSUM")
                        for j in range(4):
                            dc = half * 4 + j
                            nc.tensor.matmul(
                                tr_ps[:, j * P:(j + 1) * P],
                                x_sb[:, dc * P:(dc + 1) * P],
                                ident[:],
                                is_transpose=True,
                                start=(j == 0),
                                stop=(j == 3),
                            )
                        xt_sb = r1xt.tile([P, 4 * P], F32)
                        nc.vector.tensor_copy(xt_sb[:], tr_ps[:])
                        for j in range(4):
                            dc = half * 4 + j
                            nc.tensor.matmul(
                                lg_ps[:],
                                xt_sb[:, j * P:(j + 1) * P],
                                wg_sb[:, dc, :],
                                start=(dc == 0),
                                stop=(dc == DC - 1),
                            )
                    nc.vector.tensor_copy(lgT_sb[:, i, :], lg_ps[:])
                if dp_logits:
                    nc.sync.dma_start(
                        lg_slice[:], lgT_sb[:].rearrange("p a e -> p (a e)")
                    )

            # ---------------- logits exchange + reload --------------------------
            if dp_logits:
                nc.gpsimd.collective_compute(
                    "AllGather",
                    OP.bypass,
                    replica_groups=[list(range(E))],
                    ins=[lg_slice[:].opt()],
                    outs=[lg_full[:].opt()],
                )
                nc.sync.dma_start(
                    logits_sb[:].rearrange("p (j a) e -> p j (a e)", j=E),
                    lg_full[:].rearrange("(j p) c -> p j c", p=P),
                )
            if debug:
                nc.sync.dma_start(dbg_logits[:], logits_sb[:])

            # ---------------- phase R2: top-2 + weights (token-tile layout) -----
            with (
                tc.tile_pool(name="r2", bufs=1) as r2,
                tc.tile_pool(name="r3ps", bufs=1, space="PSUM") as r3ps,
            ):
                m1 = r2.tile([P, NT], F32)
                nc.vector.tensor_reduce(m1[:], logits_sb[:], axis=mybir.AxisListType.X, op=OP.max)
                oh1 = r2.tile([P, NT, E], F32)
                nc.vector.tensor_tensor(
                    oh1[:], logits_sb[:], m1[:].rearrange("p t -> p t ()").to_broadcast([P, NT, E]),
                    op=OP.is_equal,
                )
                masked = r2.tile([P, NT, E], F32)
                nc.vector.tensor_scalar(masked[:], oh1[:], -1e9, None, op0=OP.mult)
                nc.vector.tensor_tensor(masked[:], masked[:], logits_sb[:], op=OP.add)
                m2 = r2.tile([P, NT], F32)
                nc.vector.tensor_reduce(m2[:], masked[:], axis=mybir.AxisListType.X, op=OP.max)
                oh2 = r2.tile([P, NT, E], F32)
                nc.vector.tensor_tensor(
                    oh2[:], masked[:], m2[:].rearrange("p t -> p t ()").to_broadcast([P, NT, E]),
                    op=OP.is_equal,
                )
                delta = r2.tile([P, NT], F32)
                nc.vector.tensor_tensor(delta[:], m2[:], m1[:], op=OP.subtract)
                w1 = r2.tile([P, NT], F32)
                nc.scalar.activation(w1[:], delta[:], AF.Sigmoid, scale=-1.0)
                w2 = r2.tile([P, NT], F32)
                nc.scalar.activation(w2[:], delta[:], AF.Sigmoid)

                # select this core's expert column: oh_e = sum_E(oh * sel)
                sel_b = sel_sb[:].rearrange("p e -> p () e").to_broadcast([P, NT, E])
                tmp = r2.tile([P, NT, E], F32)
                oh1e = r2.tile([P, NT], F32)
                nc.vector.tensor_tensor(tmp[:], oh1[:], sel_b, op=OP.mult)
                nc.vector.tensor_reduce(oh1e[:], tmp[:], axis=mybir.AxisListType.X, op=OP.max)
                oh2e = r2.tile([P, NT], F32)
                nc.vector.tensor_tensor(tmp[:], oh2[:], sel_b, op=OP.mult)
                nc.vector.tensor_reduce(oh2e[:], tmp[:], axis=mybir.AxisListType.X, op=OP.max)

                # ---------------- phase R3: capacity cumsum in [tile, token] layout
                ohT_ps = r3ps.tile([P, 2 * P], F32, space="PSUM")
                nc.tensor.matmul(ohT_ps[0:NT, 0:P], oh1e[:], ident[:], is_transpose=True, start=True, stop=False)
                nc.tensor.matmul(ohT_ps[0:NT, P:2 * P], oh2e[:], ident[:], is_transpose=True, start=False, stop=True)
                oh_ic = r2.tile([NT, 2, P], F32)
                nc.vector.tensor_copy(oh_ic[:], ohT_ps[0:NT, :].rearrange("a (k p) -> a k p", k=2))

                ic = r2.tile([NT, 2, P], F32)   # per-tile inclusive cumsums, both ranks
                nc.vector.tensor_tensor_scan(
                    ic[:, 0, :], oh_ic[:, 0, :], oh_ic[:, 0, :], 0.0, op0=OP.add, op1=OP.bypass
                )
                nc.vector.tensor_tensor_scan(
                    ic[:, 1, :], oh_ic[:, 1, :], oh_ic[:, 1, :], 0.0, op0=OP.add, op1=OP.bypass
                )
                # tile totals -> [1, 64] via transpose, prefix-scan, back
                sT_ps = r3ps.tile([P, 2 * NT], F32, space="PSUM")
                nc.tensor.matmul(sT_ps[0:1, 0:NT], ic[:, 0, P - 1:P], ident[0:NT, 0:NT], is_transpose=True, start=True, stop=False)
                nc.tensor.matmul(sT_ps[0:1, NT:2 * NT], ic[:, 1, P - 1:P], ident[0:NT, 0:NT], is_transpose=True, start=False, stop=True)
                sT = r2.tile([1, 2, NT], F32)
                nc.vector.tensor_copy(sT[:], sT_ps[0:1, :].rearrange("a (k t) -> a k t", k=2))
                S1 = r2.tile([1, 2, NT], F32)
                nc.vector.tensor_tensor_scan(
                    S1[:, 0, :], sT[:, 0, :], sT[:, 0, :], 0.0, op0=OP.add, op1=OP.bypass
                )
                c0 = r2.tile([1, 1], F32)
                nc.vector.tensor_scalar(c0[:], S1[:, 0, NT - 1:NT], 2048.0, None, op0=OP.min)
                nc.vector.tensor_tensor_scan(
                    S1[:, 1, :], sT[:, 1, :], sT[:, 1, :], c0[:], op0=OP.add, op1=OP.bypass
                )
                offsT = r2.tile([1, 2, NT], F32)
                nc.vector.tensor_tensor(offsT[:], S1[:], sT[:], op=OP.subtract)
                # back-transpose offsets to [64, 1] per rank
                offs = r2.tile([NT, 2, 1], F32)
                for r in range(2):
                    offs_ps = r3ps.tile([P, 1], F32, space="PSUM", name="offs_ps")
                    nc.tensor.matmul(offs_ps[0:NT, :], offsT[:, r, :], ident[0:1, 0:1], is_transpose=True, start=True, stop=True)
                    nc.vector.tensor_copy(offs[:, r, :], offs_ps[0:NT, :])

                cs = r2.tile([NT, 2, P], F32)
                nc.vector.tensor_scalar(cs[:, 0, :], ic[:, 0, :], offs[:, 0, :], None, op0=OP.add)
                nc.vector.tensor_scalar(cs[:, 1, :], ic[:, 1, :], offs[:, 1, :], None, op0=OP.add)

                keep = r2.tile([NT, 2, P], F32)
                nc.vector.tensor_scalar(keep[:], cs[:], float(C), None, op0=OP.is_le)
                k12 = r2.tile([NT, 2, P], F32)
                nc.vector.tensor_tensor(k12[:], keep[:], oh_ic[:], op=OP.mult)

                # pos = k1*cs1 + k2*cs2 + TRASH + (-1 - TRASH)*(k1+k2)
                kcs = r2.tile([NT, 2, P], F32)
                nc.vector.tensor_tensor(kcs[:], k12[:], cs[:], op=OP.mult)
                pos_ic = r2.tile([NT, P], F32)
                nc.vector.tensor_tensor(pos_ic[:], kcs[:, 0, :], kcs[:, 1, :], op=OP.add)
                ksum = r2.tile([NT, P], F32)
                nc.vector.tensor_tensor(ksum[:], k12[:, 0, :], k12[:, 1, :], op=OP.add)
                nc.vector.tensor_scalar(
                    ksum[:], ksum[:], -float(TRASH_SLOT + 1), float(TRASH_SLOT), op0=OP.mult, op1=OP.add
                )
                nc.vector.tensor_tensor(pos_ic[:], pos_ic[:], ksum[:], op=OP.add)

                # back to token layout: pos [128, 64] (int16) and k1/k2 [128, 64]
                pk_ps = r3ps.tile([P, 3 * NT], F32, space="PSUM")
                nc.tensor.matmul(pk_ps[:, 0:NT], pos_ic[:], ident[0:NT, 0:NT], is_transpose=True, start=True, stop=False)
                nc.tensor.matmul(pk_ps[:, NT:2 * NT], k12[:, 0, :], ident[0:NT, 0:NT], is_transpose=True, start=False, stop=False)
                nc.tensor.matmul(pk_ps[:, 2 * NT:3 * NT], k12[:, 1, :], ident[0:NT, 0:NT], is_transpose=True, start=False, stop=True)
                pos_i16 = r2.tile([P, NT], I16)
                nc.vector.tensor_copy(pos_i16[:], pk_ps[:, 0:NT])
                cw_tok = r2.tile([P, NT], F32)
                t1 = r2.tile([P, NT], F32)
                nc.vector.tensor_tensor(cw_tok[:], w1[:], pk_ps[:, NT:2 * NT], op=OP.mult)
                nc.vector.tensor_tensor(t1[:], w2[:], pk_ps[:, 2 * NT:3 * NT], op=OP.mult)
                nc.vector.tensor_tensor(cw_tok[:], cw_tok[:], t1[:], op=OP.add)
                if debug:
                    nc.sync.dma_start(dbg_cw[:], cw_tok[:])
                    pos_f_dbg = r2.tile([P, NT], F32)
                    nc.vector.tensor_copy(pos_f_dbg[:], pk_ps[:, 0:NT])
                    nc.sync.dma_start(dbg_pos[:], pos_f_dbg[:])

                # ---------------- build wrapped-16 idx for the staging scatter ----
                idx_pos = persist.tile([P, NT, E], I16)   # [128, 512] wrapped: col = tile*8+g
                sh_pos = r2.tile([P, NT], I16)
                mask = [(i + 16) % 32 for i in range(32)]
                nc.vector.stream_shuffle(sh_pos[:], pos_i16[:], mask)
                for g in range(8):
                    q, lower = g // 2, (g % 2 == 0)
                    src = pos_i16 if lower else sh_pos
                    nc.vector.tensor_copy(idx_pos[0:16, :, g], src[q * 32:q * 32 + 16, :])
                for k in range(1, 8):
                    nc.sync.dma_start(idx_pos[16 * k:16 * (k + 1), :, :], idx_pos[0:16, :, :])

                # staging scatter input: rows [token_id+1, cw, 0...]
                stage_f = r2.tile([P, NT, 64], F32)
                nc.vector.memset(stage_f[:], 0.0)
                ids = r2.tile([P, NT], I32)
                nc.gpsimd.iota(ids[:], pattern=[[P, NT]], base=1, channel_multiplier=1)
                nc.vector.tensor_copy(stage_f[:, :, 0], ids[:])
                nc.vector.tensor_copy(stage_f[:, :, 1], cw_tok[:])
                for k4 in range(4):
                    nc.gpsimd.dma_scatter_add(
                        out_ap=map_stage[:],
                        in_ap=stage_f[:, 16 * k4:16 * (k4 + 1), :],
                        idxs_ap=idx_pos[:, 16 * k4:16 * (k4 + 1), :].rearrange("p a b -> p (a b)"),
                        num_idxs=T // 4, num_idxs_reg=T // 4, elem_size=64,
                        queue_num=next_q(),
                    )

            # ---------------- slot-order maps ----------------
            with tc.tile_pool(name="mapb", bufs=1) as mapb:
                map_got = persist.tile([P, C // P, 64], F32)
                # NOTE: dma_gather with num_idxs=2048 in one call faults the
                # device (ucode limit) — keep per-call idx count at 256.
                for k8 in range(8):
                    nc.gpsimd.dma_gather(
                        out_ap=map_got[:, 2 * k8:2 * (k8 + 1), :],
                        in_ap=map_stage[:],
                        idxs_ap=gidx_sb[:, 16 * k8:16 * (k8 + 1)],
                        num_idxs=C // 8, num_idxs_reg=C // 8, elem_size=64,
                        queue_num=next_q(),
                    )
                if debug:
                    nc.sync.dma_start(dbg_map[:], map_got[:])
                tok0 = mapb.tile([P, C // P], F32)
                nc.vector.tensor_scalar(tok0[:], map_got[:, :, 0], -1.0, None, op0=OP.add)
                xg_f = mapb.tile([P, C // P], F32)
                nc.vector.tensor_scalar(xg_f[:], tok0[:], 0.0, None, op0=OP.max)
                neg = mapb.tile([P, C // P], F32)
                nc.vector.tensor_scalar(neg[:], tok0[:], 0.0, None, op0=OP.is_lt)
                sc_f = mapb.tile([P, C // P], F32)
                nc.vector.tensor_scalar(sc_f[:], neg[:], float(TRASH_TOK + 1), None, op0=OP.mult)
                nc.vector.tensor_tensor(sc_f[:], sc_f[:], tok0[:], op=OP.add)
                xg_i = mapb.tile([P, C // P], I16)
                nc.vector.tensor_copy(xg_i[:], xg_f[:])
                sc_i = mapb.tile([P, C // P], I16)
                nc.vector.tensor_copy(sc_i[:], sc_f[:])

                idx_xg = persist.tile([P, C // P, E], I16)
                idx_sc = persist.tile([P, C // P, E], I16)
                mask = [(i + 16) % 32 for i in range(32)]
                sh_xg = mapb.tile([P, C // P], I16)
                nc.vector.stream_shuffle(sh_xg[:], xg_i[:], mask)
                sh_sc = mapb.tile([P, C // P], I16)
                nc.vector.stream_shuffle(sh_sc[:], sc_i[:], mask)
                for g in range(8):
                    q, lower = g // 2, (g % 2 == 0)
                    nc.vector.tensor_copy(idx_xg[0:16, :, g], (xg_i if lower else sh_xg)[q * 32:q * 32 + 16, :])
                    nc.vector.tensor_copy(idx_sc[0:16, :, g], (sc_i if lower else sh_sc)[q * 32:q * 32 + 16, :])
                for k in range(1, 8):
                    nc.sync.dma_start(idx_xg[16 * k:16 * (k + 1), :, :], idx_xg[0:16, :, :])
                    nc.sync.dma_start(idx_sc[16 * k:16 * (k + 1), :, :], idx_sc[0:16, :, :])

            # ---------------- dispatch (row gather + PE transpose) + FFN --------
            with (
                tc.tile_pool(name="gpool", bufs=2) as gpool,
                tc.tile_pool(name="xTep", bufs=2) as xTep,
                tc.tile_pool(name="tpps", bufs=2, space="PSUM") as tpps,
                tc.tile_pool(name="hT", bufs=3) as hTp,
                tc.tile_pool(name="ypool", bufs=2) as ypool,
                tc.tile_pool(name="hps", bufs=2, space="PSUM") as hps,
                tc.tile_pool(name="yps", bufs=1, space="PSUM") as yps,
            ):
                def emit_gather(b):
                    xg = gpool.tile([P, 2, D], F32, name="xg")
                    nc.gpsimd.dma_gather(
                        out_ap=xg[:],
                        in_ap=x_in[:],
                        idxs_ap=idx_xg[:, 2 * b:2 * (b + 1), :].rearrange("p a b -> p (a b)"),
                        num_idxs=BS, num_idxs_reg=BS, elem_size=D,
                        queue_num=next_q(),
                    )
                    return xg

                def emit_transpose(xg):
                    xTe = xTep.tile([P, DC, BS], BF16, name="xTe")
                    for st in range(2):
                        for gg in range(2):
                            tp = tpps.tile([P, 4, P], F32, space="PSUM", name="tp")
                            for j in range(4):
                                dc = gg * 4 + j
                                nc.tensor.matmul(
                                    tp[:, j, :],
                                    xg[:, st, dc * P:(dc + 1) * P],
                                    ident[:],
                                    is_transpose=True,
                                    start=(j == 0),
                                    stop=(j == 3),
                                )
                            nc.vector.tensor_copy(
                                xTe[:, gg * 4:(gg + 1) * 4, st * P:(st + 1) * P], tp[:]
                            )
                    return xTe

                xg_tiles = {0: emit_gather(0), 1: emit_gather(1)}
                xTe_cur = emit_transpose(xg_tiles.pop(0))

                for b in range(NB):
                    if b + 2 < NB:
                        xg_tiles[b + 2] = emit_gather(b + 2)
                    xTe_next = emit_transpose(xg_tiles.pop(b + 1)) if b + 1 < NB else None

                    y_tiles = [
                        [yps.tile([P, 512], F32, space="PSUM", name=f"y{st}{dg}") for dg in range(2)]
                        for st in range(2)
                    ]
                    h_prev = None
                    for hc in range(HC):
                        h_ps = hps.tile([P, BS], F32, space="PSUM", name="hps")
                        for dc in range(DC):
                            nc.tensor.matmul(
                                h_ps[:],
                                w1_sb[:, dc, hc * P:(hc + 1) * P],
                                xTe_cur[:, dc, :],
                                start=(dc == 0),
                                stop=(dc == DC - 1),
                            )
                        h_sb = hTp.tile([P, BS], BF16, name="hsb")
                        nc.scalar.activation(h_sb[:], h_ps[:], AF.Gelu_apprx_tanh, bias=b1_sb[:, hc:hc + 1])
                        if h_prev is not None:
                            ph, phc = h_prev
                            for st in range(2):
                                for dg in range(2):
                                    nc.tensor.matmul(
                                        y_tiles[st][dg][:],
                                        ph[:, st * P:(st + 1) * P],
                                        w2_sb[:, phc, dg * 512:(dg + 1) * 512],
                                        start=(phc == 0),
                                        stop=False,
                                    )
                        h_prev = (h_sb, hc)
                    ph, phc = h_prev
                    for st in range(2):
                        for dg in range(2):
                            nc.tensor.matmul(
                                y_tiles[st][dg][:],
                                ph[:, st * P:(st + 1) * P],
                                w2_sb[:, phc, dg * 512:(dg + 1) * 512],
                                start=False,
                                stop=False,
                            )
                    # bias add via ones-outer-product, then weighted drain
                    y_sb = ypool.tile([P, 2, D], BF16, name="ysb")
                    for st in range(2):
                        tile_idx = b * 2 + st
                        for dg in range(2):
                            nc.tensor.matmul(
                                y_tiles[st][dg][:],
                                ones1[:],
                                b2_sb[:, dg * 512:(dg + 1) * 512],
                                start=False,
                                stop=True,
                            )
                            if st == 0:
                                nc.scalar.activation(
                                    y_sb[:, st, dg * 512:(dg + 1) * 512],
                                    y_tiles[st][dg][:],
                                    AF.Copy,
                                    scale=map_got[:, tile_idx, 1:2],
                                )
                            else:
                                nc.vector.tensor_scalar(
                                    y_sb[:, st, dg * 512:(dg + 1) * 512],
                                    y_tiles[st][dg][:],
                                    map_got[:, tile_idx, 1:2],
                                    None,
                                    op0=OP.mult,
                                )
                    nc.gpsimd.dma_scatter_add(
                        out_ap=partial[:], in_ap=y_sb[:],
                        idxs_ap=idx_sc[:, 2 * b:2 * (b + 1), :].rearrange("p a b -> p (a b)"),
                        num_idxs=BS, num_idxs_reg=BS, elem_size=D,
                        queue_num=next_q(),
                    )
                    xTe_cur = xTe_next

            # ---------------- combine: ReduceScatter + output ----------------
            nc.gpsimd.collective_compute(
                "ReduceScatter",
                OP.add,
                replica_groups=[list(range(E))],
                ins=[partial[0:T, :].opt()],
                outs=[rs_out[:].opt()],
            )
            with tc.tile_pool(name="outp", bufs=2) as outp:
                for i in range(T // E // P):
                    t_bf = outp.tile([P, D], BF16)
                    nc.sync.dma_start(t_bf[:], rs_out[i * P:(i + 1) * P, :])
                    t_f = outp.tile([P, D], F32)
                    nc.vector.tensor_copy(t_f[:], t_bf[:])
                    nc.sync.dma_start(out_sl[i * P:(i + 1) * P, :], t_f[:])

    nc.compile()
    # Post-schedule fixup: DMASW semaphore lanes are assigned round-robin
    # (mod 8) over Pool-engine DMA instructions in final scheduled order, and
    # each lane is locked to one SWDGE queue. Assign queue = lane % 4 so all
    # 4 queues are used consistently.
    cnt = 0
    for bb in nc.m.functions[0].blocks:
        for inst in bb.instructions:
            tn = type(inst).__name__
            if inst.engine == mybir.EngineType.Pool and ("DMA" in tn or "Dma" in tn):
                inst.queue_num = (cnt % 8) % 4
                cnt += 1
    return nc


_NC_CACHE = {}


def _get_nc():
    if "nc" not in _NC_CACHE:
        _NC_CACHE["nc"] = build_moe()
    return _NC_CACHE["nc"]


def make_inputs(x, Wg, W1, b1, W2, b2):
    """Host-side sharding: per-core input maps."""
    bf = ml_dtypes.bfloat16
    x = np.ascontiguousarray(np.asarray(x, dtype=np.float32).reshape(T, D))
    wg = np.ascontiguousarray(
        np.asarray(Wg, dtype=np.float32).reshape(DC, P, E).transpose(1, 0, 2)
    )
    gidx = wrap16_const(C)
    in_maps = []
    for e in range(E):
        w1s = np.ascontiguousarray(
            np.asarray(W1[e], dtype=np.float32).reshape(DC, P, H).transpose(1, 0, 2).astype(bf)
        )
        w2s = np.ascontiguousarray(
            np.asarray(W2[e], dtype=np.float32).reshape(HC, P, D).transpose(1, 0, 2).astype(bf)
        )
        b1s = np.ascontiguousarray(np.asarray(b1[e], dtype=np.float32).reshape(HC, P).T)
        b2r = np.asarray(b2[e], dtype=np.float32).reshape(1, D).astype(bf)
        sel = np.zeros((P, E), dtype=np.float32)
        sel[:, e] = 1.0
        in_maps.append({
            "x": x, "xsl": x[e * (T // E):(e + 1) * (T // E)], "wg": wg, "sel": sel,
            "w1s": w1s, "w2s": w2s, "b1s": b1s, "b2r": b2r,
            "gidx": gidx,
        })
    return in_maps


def kernel(x, Wg, W1, b1, W2, b2):
    nc = _get_nc()
    in_maps = make_inputs(x, Wg, W1, b1, W2, b2)
    res = run_bass_kernel_spmd(nc, in_maps, list(range(E)))
    out = np.concatenate([res.results[e]["out_slice"] for e in range(E)], axis=0)
    return out.reshape(B, S, D).astype(np.float32)
